# revision 12
# baseline (speedup 1.0000x reference)
"""AntiSymmetricDGN on 8 TRN2 NeuronCores (Bass/Tile, SPMD) — V2.

Node-sharded graph parallel; per-core state TRANSPOSED in SBUF
[feat(partitions), nodes(free)].

Per conv iteration:
  - table hw^T = gcn_w^T @ h^T computed per superblock, transposed to
    node-major bf16 rows (persistent rows_all), AllGathered into
    replicated HBM half-tables A/B [25000, 128] bf16. AG-A fires after
    superblock 6 is staged, AG-B after superblock 12, so the collective
    overlaps the previous iteration's gather tail.
  - edge gather: SWDGE dma_gather of 256B rows by src index; chunks of
    128 edges grouped per (128-col dst window, table half); capacities
    are max-over-cores (uniform SPMD stream); pads gather row 0.
    Self-loops are NOT gathered.
  - scatter/segment-sum: per chunk, a [128,128] one-hot*norm S matrix is
    generated on DVE (iota == colidx) * val, then TensorE accumulates
    psy[:, win] += g_chunk.T @ S. Self-loop term added via diagonal
    matmuls: psy += rows_all[tile].T @ diag(dinv^2).
  - dense ops (aW, tanh, update, staging) interleaved per superblock.
"""
import math
import os
import numpy as np

import concourse.bass as bass
from concourse import mybir, bacc
from concourse.bass_utils import run_bass_kernel_spmd
from concourse.tile import TileContext

# problem constants
N, E, IN, H, H2, OUT = 50000, 600000, 256, 128, 64, 40
EPS, GAMMA = 0.1, 0.1
NCORES = 8
NSH = N // NCORES          # 6250 nodes per core
WCOL = 128                 # dst window width (S chunk width)
NW = (NSH + WCOL - 1) // WCOL      # 49 windows per core (last = 106 cols)
SBW = 4                    # windows per superblock (512 dst cols)
NSB = (NW + SBW - 1) // SBW        # 13 superblocks
HALF = N // 2              # table half split for int16 gather indices
CHUNK = 128
NTILE = (NSH + 127) // 128         # 49 node tiles per core

F32 = mybir.dt.float32
BF16 = mybir.dt.bfloat16
I16 = mybir.dt.int16
AF = mybir.ActivationFunctionType
ALU = mybir.AluOpType


def _wslice(s):
    lo = s * SBW * WCOL
    hi = min(NSH, (s + 1) * SBW * WCOL)
    return lo, hi - lo


# ----------------------------------------------------------------- prep
def _prep_graph(edge_index):
    src = np.asarray(edge_index[0], dtype=np.int64)
    dst = np.asarray(edge_index[1], dtype=np.int64)
    # GCN norm with self loops included in degrees
    deg = np.bincount(dst, minlength=N).astype(np.float32) + 1.0
    dinv = (1.0 / np.sqrt(deg)).astype(np.float32)
    norm = (dinv[src] * dinv[dst]).astype(np.float32)
    dinv2 = (dinv * dinv).astype(np.float32)       # self-loop coefficients

    core = dst // NSH
    col = dst % NSH
    win = col // WCOL
    src_core = src // NSH
    src_r = src % NSH
    half = (src_r >= NSH // 2).astype(np.int64)
    # half-tables are per-core concatenations: tableA row = core*3125 + r
    tabrow = src_core * (NSH // 2) + (src_r - half * (NSH // 2))

    key = (core * NW + win) * 2 + half
    order = np.argsort(key, kind="stable")          # natural (random) src order kept
    srcS, colS, normS, keyS = tabrow[order], col[order], norm[order], key[order]

    cnt = np.bincount(key, minlength=NCORES * NW * 2).reshape(NCORES, NW, 2)
    CW = np.ceil(cnt / CHUNK).astype(np.int64).max(axis=0)   # [NW, 2] chunks per group
    assert CW.max() * CHUNK <= 1024, CW.max()

    # chunk index layout: for s in sb: for h in (0,1): for w in windows(s): CW[w,h] chunks
    sb_windows = [list(range(s * SBW, min((s + 1) * SBW, NW))) for s in range(NSB)]
    choff_sb = []              # first chunk id of each sb
    blkoff = {}                # (s,h,w) -> block offset within sb (S tile index)
    hblkoff = {}               # (s,h,w) -> block offset within the half's g tile
    ch = 0
    for s, ws in enumerate(sb_windows):
        choff_sb.append(ch)
        boff = 0
        for h in (0, 1):
            hoff = 0
            for w in ws:
                blkoff[(s, h, w)] = boff
                hblkoff[(s, h, w)] = hoff
                nch = int(CW[w, h])
                ch += nch
                boff += nch
                hoff += nch
    TOTCH = ch
    C_sb = [int(sum(CW[w, 0] + CW[w, 1] for w in ws)) for ws in sb_windows]
    CH_sb = [[int(sum(CW[w, h] for w in ws)) for ws in sb_windows] for h in (0, 1)]
    CSB_MAX = max(C_sb)
    CHA_MAX = max(CH_sb[0])
    CHB_MAX = max(CH_sb[1])
    TOT = TOTCH * CHUNK        # gather slots per core per iteration

    grp_start = np.concatenate([[0], np.cumsum(cnt.reshape(-1))]).astype(np.int64)

    idx = np.zeros((NCORES, TOT), dtype=np.int16)
    scoef = np.zeros((NCORES, CHUNK, TOTCH, WCOL), dtype=mybir.dt.np(BF16))

    for c in range(NCORES):
        off = 0
        for s, ws in enumerate(sb_windows):
            for h in (0, 1):
                for w in ws:
                    g = (c * NW + w) * 2 + h
                    st, n = grp_start[g], int(cnt[c, w, h])
                    e_src = srcS[st:st + n]
                    e_col = colS[st:st + n]
                    e_nrm = normS[st:st + n]
                    cap = int(CW[w, h])
                    for k in range(cap):
                        lo = k * CHUNK
                        m = max(0, min(CHUNK, n - lo))
                        gch = None
                        # global chunk id for (s,h,w,k)
                        gch = choff_sb[s] + blkoff[(s, h, w)] + k
                        if m > 0:
                            idx[c, off:off + m] = e_src[lo:lo + m]
                            rel = (e_col[lo:lo + m] - w * WCOL).astype(np.int64)
                            scoef[c, np.arange(m), gch, rel] = e_nrm[lo:lo + m]
                        off += CHUNK
        assert off == TOT

    def wrap_idx(flat):
        L = len(flat)
        assert L % 16 == 0
        w16 = flat.reshape(L // 16, 16).T.copy()     # [16, L/16]
        return np.tile(w16, (8, 1))                   # [128, L/16]

    # per-core diagonal chunks: D[n, c] = dinv2[core*NSH + t*128 + n] if n==c
    diagc = np.zeros((NCORES, CHUNK, NTILE, CHUNK), dtype=np.float32)
    for c in range(NCORES):
        for t in range(NTILE):
            tw = min(128, NSH - t * 128)
            v = dinv2[c * NSH + t * 128: c * NSH + t * 128 + tw]
            diagc[c, np.arange(tw), t, np.arange(tw)] = v

    meta = dict(CW=CW, sb_windows=sb_windows, choff_sb=choff_sb, blkoff=blkoff,
                hblkoff=hblkoff, C_sb=C_sb, CSB_MAX=CSB_MAX,
                CHA_MAX=CHA_MAX, CHB_MAX=CHB_MAX, TOTCH=TOTCH, TOT=TOT)
    bf = mybir.dt.np(BF16)
    percore = []
    for c in range(NCORES):
        percore.append(dict(
            idx=wrap_idx(idx[c]).astype(np.int16),
            scoef=scoef[c],
            diagc=diagc[c].astype(bf),
        ))
    return meta, percore


# ---------------------------------------------------------------- build
def _build(meta):
    CW = meta["CW"]
    sb_windows = meta["sb_windows"]
    choff_sb, blkoff = meta["choff_sb"], meta["blkoff"]
    hblkoff = meta["hblkoff"]
    C_sb, CSB_MAX, TOTCH, TOT = meta["C_sb"], meta["CSB_MAX"], meta["TOTCH"], meta["TOT"]
    CHA_MAX, CHB_MAX = meta["CHA_MAX"], meta["CHB_MAX"]

    nc = bacc.Bacc(num_devices=NCORES, num_swdge_queues=4)
    p_xT = nc.declare_dram_parameter("xT", [IN, NSH], F32, isOutput=False)
    p_idx = nc.declare_dram_parameter("idx", [128, TOT // 16], I16, isOutput=False)
    p_sc = nc.declare_dram_parameter("scoef", [128, TOTCH, WCOL], BF16, isOutput=False)
    p_diag = nc.declare_dram_parameter("diagc", [128, NTILE, 128], BF16, isOutput=False)
    p_w0T = nc.declare_dram_parameter("w0T", [IN, H], F32, isOutput=False)
    p_b0 = nc.declare_dram_parameter("b0", [H, 1], F32, isOutput=False)
    p_aW1T = nc.declare_dram_parameter("aW1T", [H, H], F32, isOutput=False)
    p_gw1 = nc.declare_dram_parameter("gw1", [H, H], F32, isOutput=False)
    p_ba1 = nc.declare_dram_parameter("ba1", [H, 1], F32, isOutput=False)
    p_w2T = nc.declare_dram_parameter("w2T", [H, H2], F32, isOutput=False)
    p_b2 = nc.declare_dram_parameter("b2", [H2, 1], F32, isOutput=False)
    p_aW2T = nc.declare_dram_parameter("aW2T", [H2, H2], F32, isOutput=False)
    p_gw2 = nc.declare_dram_parameter("gw2", [H2, H2], F32, isOutput=False)
    p_ba2 = nc.declare_dram_parameter("ba2", [H2, 1], F32, isOutput=False)
    p_wfT = nc.declare_dram_parameter("wfT", [H2, OUT], F32, isOutput=False)
    p_bfc = nc.declare_dram_parameter("bfc", [128, OUT], F32, isOutput=False)
    p_ident = nc.declare_dram_parameter("ident", [128, 128], BF16, isOutput=False)
    p_out = nc.declare_dram_parameter("out", [NSH, OUT], F32, isOutput=True)
    p_hd = nc.declare_dram_parameter("hdump", [H, NSH], F32, isOutput=True)
    K_DUMP = os.environ.get("K_DUMP", "")

    ag_ins = [nc.dram_tensor(f"ag_in{i}", [NSH, H], BF16) for i in range(4)]
    tablesA = [nc.dram_tensor(f"tableA{i}", [HALF, H], BF16, addr_space="Shared")
               for i in range(4)]
    tablesB = [nc.dram_tensor(f"tableB{i}", [HALF, H], BF16, addr_space="Shared")
               for i in range(4)]

    with TileContext(nc) as tc:
        with (
            tc.tile_pool(name="const", bufs=1) as cp,
            tc.tile_pool(name="xin", bufs=2) as xp,
            tc.tile_pool(name="gatA", bufs=5) as gpa,
            tc.tile_pool(name="gatB", bufs=3) as gpb,
            tc.tile_pool(name="sstr", bufs=2) as sp,
            tc.tile_pool(name="wrk", bufs=2) as wp,
            tc.tile_pool(name="pa", bufs=3, space="PSUM") as pa,
            tc.tile_pool(name="pt", bufs=2, space="PSUM") as pt,
            tc.tile_pool(name="py", bufs=3, space="PSUM") as py,
        ):
            # ---- persistent state + constants
            hT = cp.tile([H, NSH], F32, tag="hT")
            h2T = cp.tile([H2, NSH], F32, tag="h2T")
            hwT = cp.tile([H, NSH], BF16, tag="hwT")
            rows_all = cp.tile([128, NTILE, 128], BF16, tag="rows_all")
            t_idx = cp.tile([128, TOT // 16], I16, tag="idx")
            t_diag = cp.tile([128, NTILE, 128], BF16, tag="diagc")
            w0a = cp.tile([128, H], F32, tag="w0a")
            w0b = cp.tile([128, H], F32, tag="w0b")
            b0 = cp.tile([H, 1], F32, tag="b0")
            aW1T = cp.tile([H, H], F32, tag="aW1T")
            gw1 = cp.tile([H, H], F32, tag="gw1")
            ba1 = cp.tile([H, 1], F32, tag="ba1")
            w2T = cp.tile([H, H2], F32, tag="w2T")
            b2 = cp.tile([H2, 1], F32, tag="b2")
            aW2T = cp.tile([H2, H2], F32, tag="aW2T")
            gw2 = cp.tile([H2, H2], F32, tag="gw2")
            ba2 = cp.tile([H2, 1], F32, tag="ba2")
            wfT = cp.tile([H2, OUT], F32, tag="wfT")
            bfc = cp.tile([128, OUT], F32, tag="bfc")
            ident = cp.tile([128, 128], BF16, tag="ident")

            nc.sync.dma_start(out=t_idx[:], in_=p_idx[:, :])
            nc.sync.dma_start(out=t_diag[:, :, :], in_=p_diag[:, :, :])
            nc.sync.dma_start(out=w0a[:], in_=p_w0T[0:128, :])
            nc.sync.dma_start(out=w0b[:], in_=p_w0T[128:256, :])
            nc.sync.dma_start(out=b0[:], in_=p_b0[:, :])
            nc.sync.dma_start(out=aW1T[:], in_=p_aW1T[:, :])
            nc.sync.dma_start(out=gw1[:], in_=p_gw1[:, :])
            nc.sync.dma_start(out=ba1[:], in_=p_ba1[:, :])
            nc.sync.dma_start(out=w2T[:], in_=p_w2T[:, :])
            nc.sync.dma_start(out=b2[:], in_=p_b2[:, :])
            nc.sync.dma_start(out=aW2T[:], in_=p_aW2T[:, :])
            nc.sync.dma_start(out=gw2[:], in_=p_gw2[:, :])
            nc.sync.dma_start(out=ba2[:], in_=p_ba2[:, :])
            nc.sync.dma_start(out=wfT[:], in_=p_wfT[:, :])
            nc.sync.dma_start(out=bfc[:], in_=p_bfc[:, :])
            nc.sync.dma_start(out=ident[:], in_=p_ident[:, :])

            # ------------------------------------------------ staging helpers
            def stage_sb(s, src_t, srcdim, gwt, ag_in):
                """hw^T = gwt.T @ src_t for superblock s; write hwT (bf16),
                transpose to rows_all tiles, DMA rows to ag_in."""
                lo, n = _wslice(s)
                ps = pa.tile([srcdim, 512], F32, tag="pa")
                nc.tensor.matmul(ps[:, :n], gwt[:], src_t[:, lo:lo + n],
                                 start=True, stop=True)
                nc.scalar.activation(hwT[0:srcdim, lo:lo + n], ps[:, :n], AF.Copy)
                nt = (n + 127) // 128
                for t in range(nt):
                    tile = s * SBW + t
                    tw = min(128, n - t * 128)
                    ptt = pt.tile([128, 128], BF16, tag="pt")
                    nc.tensor.transpose(ptt[:tw, :srcdim],
                                        hwT[0:srcdim, lo + t * 128: lo + t * 128 + tw],
                                        ident[:srcdim, :srcdim])
                    nc.scalar.activation(rows_all[:tw, tile, 0:srcdim],
                                         ptt[:tw, :srcdim], AF.Copy)
                    nc.sync.dma_start(
                        out=ag_in[lo + t * 128: lo + t * 128 + tw, 0:srcdim],
                        in_=rows_all[:tw, tile, 0:srcdim])

            def fire_ag(it, part):
                if part == 0:
                    nc.gpsimd.collective_compute(
                        "AllGather", ALU.bypass,
                        replica_groups=[list(range(NCORES))],
                        ins=[ag_ins[it][0:NSH // 2, :]], outs=[tablesA[it][:, :]])
                else:
                    nc.gpsimd.collective_compute(
                        "AllGather", ALU.bypass,
                        replica_groups=[list(range(NCORES))],
                        ins=[ag_ins[it][NSH // 2:NSH, :]], outs=[tablesB[it][:, :]])

            # ------------------------------------------------ gather issue
            qn_state = [0]

            def gather_group(g_tiles, it, s, h):
                """issue gathers for (superblock s, half h)."""
                tab = (tablesA if h == 0 else tablesB)[it]
                g = g_tiles[(s, h)]
                for w in sb_windows[s]:
                    cap = int(CW[w, h])
                    if cap == 0:
                        continue
                    b0_ = hblkoff[(s, h, w)]
                    gch = choff_sb[s] + blkoff[(s, h, w)]
                    o = gch * CHUNK          # global slot offset
                    nidx = cap * CHUNK
                    nc.gpsimd.dma_gather(
                        out_ap=g[:, b0_:b0_ + cap, :], in_ap=tab[:, :],
                        idxs_ap=t_idx[:, o // 16:(o + nidx) // 16],
                        num_idxs=nidx, num_idxs_reg=nidx,
                        elem_size=H, queue_num=qn_state[0] % 4)
                    qn_state[0] += 1

            # ------------------------------------------------ psy compute
            def psy_sb(s, state_t, dim, aWt, bias_t, gA, gB):
                """accumulate psy for superblock s, then tanh+update state."""
                lo, n = _wslice(s)
                st_ = sp.tile([128, CSB_MAX, WCOL], BF16, tag="sg")
                nc.scalar.dma_start(
                    out=st_[:, 0:C_sb[s], :],
                    in_=p_sc[:, choff_sb[s]:choff_sb[s] + C_sb[s], :])
                psy = py.tile([dim, 512], F32, tag="py")
                first = True
                for h, gh in ((0, gA), (1, gB)):
                    for w in sb_windows[s]:
                        wl = w - s * SBW
                        wn = min(WCOL, NSH - w * WCOL)
                        for k in range(int(CW[w, h])):
                            blk = blkoff[(s, h, w)] + k
                            hblk = hblkoff[(s, h, w)] + k
                            nc.tensor.matmul(
                                psy[:, wl * WCOL: wl * WCOL + wn],
                                gh[:, hblk, 0:dim], st_[:, blk, 0:wn],
                                start=first, stop=False, skip_group_check=True)
                            first = False
                # self-loop diagonal terms
                nt = (n + 127) // 128
                for t in range(nt):
                    tile = s * SBW + t
                    tw = min(128, n - t * 128)
                    nc.tensor.matmul(
                        psy[:, t * 128: t * 128 + tw],
                        rows_all[0:tw, tile, 0:dim],
                        t_diag[0:tw, tile, 0:tw],
                        start=False, stop=False, skip_group_check=True)
                # aW term
                nc.tensor.matmul(psy[:, :n], aWt[:], state_t[:, lo:lo + n],
                                 start=False, stop=True, skip_group_check=True)
                upd = wp.tile([dim, 512], F32, tag="upd")
                nc.scalar.activation(upd[:, :n], psy[:, :n], AF.Tanh, bias=bias_t[:, :])
                nc.vector.scalar_tensor_tensor(
                    state_t[:, lo:lo + n], upd[:, :n], EPS,
                    state_t[:, lo:lo + n], ALU.mult, ALU.add)

            # ------------------------------------------------ final per sb
            def final_sb(s):
                lo, n = _wslice(s)
                nt = (n + 127) // 128
                for t in range(nt):
                    t0_ = lo + t * 128
                    tw = min(128, n - t * 128)
                    pf = pa.tile([128, 512], F32, tag="pa")
                    nc.tensor.matmul(pf[:tw, :OUT], h2T[:, t0_:t0_ + tw],
                                     wfT[:], start=True, stop=True)
                    lg = wp.tile([128, OUT], F32, tag="lg")
                    nc.vector.tensor_tensor(lg[:tw, :], pf[:tw, :OUT], bfc[:tw, :], ALU.add)
                    nmx = wp.tile([128, 1], F32, tag="nmx")
                    nc.vector.tensor_reduce(nmx[:tw, :], lg[:tw, :],
                                            mybir.AxisListType.X, ALU.max, negate=True)
                    ex = wp.tile([128, OUT], F32, tag="ex")
                    se = wp.tile([128, 1], F32, tag="se")
                    nc.scalar.activation(ex[:tw, :], lg[:tw, :], AF.Exp,
                                         bias=nmx[:tw, :], accum_out=se[:tw, :])
                    lse = wp.tile([128, 1], F32, tag="lse")
                    nc.scalar.activation(lse[:tw, :], se[:tw, :], AF.Ln)
                    shift = wp.tile([128, 1], F32, tag="shift")
                    nc.vector.tensor_tensor(shift[:tw, :], nmx[:tw, :], lse[:tw, :],
                                            ALU.subtract)
                    ot = wp.tile([128, OUT], F32, tag="ot")
                    nc.vector.tensor_scalar(ot[:tw, :], lg[:tw, :], shift[:tw, :],
                                            None, ALU.add)
                    nc.sync.dma_start(out=p_out[t0_:t0_ + tw, :], in_=ot[:tw, :])

            # ================================================ layer 0 (+T0 staging)
            for s in range(NSB):
                lo, n = _wslice(s)
                ps = pa.tile([H, 512], F32, tag="pa")
                for kc, w0t in enumerate((w0a, w0b)):
                    xt = xp.tile([128, 512], F32, tag="xt")
                    nc.sync.dma_start(out=xt[:, :n],
                                      in_=p_xT[kc * 128:(kc + 1) * 128, lo:lo + n])
                    nc.tensor.matmul(ps[:, :n], w0t[:], xt[:, :n],
                                     start=(kc == 0), stop=(kc == 1))
                t0_ = wp.tile([H, 512], F32, tag="upd")
                nc.scalar.activation(t0_[:, :n], ps[:, :n], AF.Identity, bias=b0[:, :])
                nc.vector.scalar_tensor_tensor(hT[:, lo:lo + n], t0_[:, :n], 0.01,
                                               t0_[:, :n], ALU.mult, ALU.max)
                stage_sb(s, hT, H, gw1, ag_ins[0])
                if s == 6:
                    fire_ag(0, 0)
            fire_ag(0, 1)
            if K_DUMP == "h0":
                nc.sync.dma_start(out=p_hd[:, :], in_=hT[:, :])

            # ================================================ conv iterations
            def conv_iteration(it, state_t, dim, aWt, bias_t, gwt_next, post):
                """post(s) runs after update of superblock s (staging for the
                next phase); gathers are pipelined A-ahead."""
                g_tiles = {}
                for s in range(NSB):
                    g_tiles[(s, 0)] = gpa.tile([128, CHA_MAX, 128], BF16, tag="ga",
                                               name=f"ga_{it}_{s}")
                    g_tiles[(s, 1)] = gpb.tile([128, CHB_MAX, 128], BF16, tag="gb",
                                               name=f"gb_{it}_{s}")
                # prologue: A gathers several superblocks ahead
                for s0 in range(min(5, NSB)):
                    gather_group(g_tiles, it, s0, 0)
                gather_group(g_tiles, it, 0, 1)
                for s in range(NSB):
                    if s + 1 < NSB:
                        gather_group(g_tiles, it, s + 1, 1)
                    if s + 5 < NSB:
                        gather_group(g_tiles, it, s + 5, 0)
                    psy_sb(s, state_t, dim, aWt, bias_t, g_tiles[(s, 0)], g_tiles[(s, 1)])
                    post(s)

            # ---- conv1 iter 0 (stage T1)
            def post0(s):
                stage_sb(s, hT, H, gw1, ag_ins[1])
                if s == 6:
                    fire_ag(1, 0)
                if s == NSB - 1:
                    fire_ag(1, 1)
            conv_iteration(0, hT, H, aW1T, ba1, gw1, post0)
            if K_DUMP == "it1":
                nc.sync.dma_start(out=p_hd[:, :], in_=hT[:, :])

            # ---- conv1 iter 1 (stage T2)
            def post1(s):
                stage_sb(s, hT, H, gw1, ag_ins[2])
                if s == 6:
                    fire_ag(2, 0)
                if s == NSB - 1:
                    fire_ag(2, 1)
            conv_iteration(1, hT, H, aW1T, ba1, gw1, post1)
            if K_DUMP == "it2":
                nc.sync.dma_start(out=p_hd[:, :], in_=hT[:, :])

            # ---- conv1 iter 2 (transition + stage T3 from h2T)
            def post2(s):
                lo, n = _wslice(s)
                gk = wp.tile([H, 512], F32, tag="gk")
                nc.vector.scalar_tensor_tensor(gk[:, :n], hT[:, lo:lo + n], 0.01,
                                               hT[:, lo:lo + n], ALU.mult, ALU.max)
                ps = pa.tile([H2, 512], F32, tag="pa")
                nc.tensor.matmul(ps[:, :n], w2T[:], gk[:, :n], start=True, stop=True)
                t2 = wp.tile([H2, 512], F32, tag="upd")
                nc.scalar.activation(t2[:, :n], ps[:, :n], AF.Identity, bias=b2[:, :])
                nc.vector.scalar_tensor_tensor(h2T[:, lo:lo + n], t2[:, :n], 0.01,
                                               t2[:, :n], ALU.mult, ALU.max)
                stage_sb(s, h2T, H2, gw2, ag_ins[3])
                if s == 6:
                    fire_ag(3, 0)
                if s == NSB - 1:
                    fire_ag(3, 1)
            conv_iteration(2, hT, H, aW1T, ba1, gw1, post2)
            if K_DUMP == "it3":
                nc.sync.dma_start(out=p_hd[:, :], in_=hT[:, :])

            # ---- conv2 (final per sb)
            conv_iteration(3, h2T, H2, aW2T, ba2, None, final_sb)
            if K_DUMP == "h2":
                nc.sync.dma_start(out=p_hd[:64, :], in_=h2T[:, :])

    nc.finalize()
    return nc


# ----------------------------------------------------------------- run
def kernel(x, edge_index, w_hid, b_hid, W_a1, gcn_w1, b_a1,
           w_hid2, b_hid2, W_a2, gcn_w2, b_a2, w_fc, b_fc, _trace=False):
    x = np.asarray(x, np.float32)
    meta, percore = _prep_graph(edge_index)
    nc = _build(meta)

    f32 = np.float32
    bf = mybir.dt.np(BF16)
    w0T = np.ascontiguousarray(np.asarray(w_hid, f32).T)            # [256,128]
    aW1 = np.asarray(W_a1, f32)
    aW1T = np.ascontiguousarray(aW1.T - aW1 - GAMMA * np.eye(H, dtype=f32))
    aW2 = np.asarray(W_a2, f32)
    aW2T = np.ascontiguousarray(aW2.T - aW2 - GAMMA * np.eye(H2, dtype=f32))
    common = dict(
        w0T=w0T,
        b0=np.asarray(b_hid, f32).reshape(H, 1),
        aW1T=aW1T,
        gw1=np.ascontiguousarray(np.asarray(gcn_w1, f32)),
        ba1=np.asarray(b_a1, f32).reshape(H, 1),
        w2T=np.ascontiguousarray(np.asarray(w_hid2, f32).T),
        b2=np.asarray(b_hid2, f32).reshape(H2, 1),
        aW2T=aW2T,
        gw2=np.ascontiguousarray(np.asarray(gcn_w2, f32)),
        ba2=np.asarray(b_a2, f32).reshape(H2, 1),
        wfT=np.ascontiguousarray(np.asarray(w_fc, f32).T),
        bfc=np.tile(np.asarray(b_fc, f32).reshape(1, OUT), (128, 1)),
        ident=np.eye(128, dtype=bf),
    )
    in_maps = []
    for c in range(NCORES):
        xT = np.ascontiguousarray(x[c * NSH:(c + 1) * NSH].T)
        in_maps.append({"xT": xT, **percore[c], **common})

    res = run_bass_kernel_spmd(nc, in_maps, list(range(NCORES)), trace=_trace)
    out = np.concatenate([res.results[c]["out"] for c in range(NCORES)], axis=0)
    kernel.last_hdump = np.stack([res.results[c]["hdump"] for c in range(NCORES)])
    kernel.last_exec_time_ns = res.exec_time_ns
    kernel.last_results = res
    return out


# revision 14
# speedup vs baseline: 1.1055x; 1.1055x over previous
"""AntiSymmetricDGN on 8 TRN2 NeuronCores (Bass/Tile, SPMD) — V2.

Node-sharded graph parallel; per-core state TRANSPOSED in SBUF
[feat(partitions), nodes(free)].

Per conv iteration:
  - table hw^T = gcn_w^T @ h^T computed per superblock, transposed to
    node-major bf16 rows (persistent rows_all), AllGathered into
    replicated HBM half-tables A/B [25000, 128] bf16. AG-A fires after
    superblock 6 is staged, AG-B after superblock 12, so the collective
    overlaps the previous iteration's gather tail.
  - edge gather: SWDGE dma_gather of 256B rows by src index; chunks of
    128 edges grouped per (128-col dst window, table half); capacities
    are max-over-cores (uniform SPMD stream); pads gather row 0.
    Self-loops are NOT gathered.
  - scatter/segment-sum: per chunk, a [128,128] one-hot*norm S matrix
    (precomputed host-side, streamed per superblock from HBM on the
    scalar HWDGE ring) is multiplied on TensorE:
    psy[:, win] += g_chunk.T @ S. Self-loop term added via diagonal
    matmuls: psy += rows_all[tile].T @ diag(dinv^2).
  - dense ops (aW, tanh, update, staging) interleaved per superblock.
"""
import math
import os
import numpy as np

import concourse.bass as bass
from concourse import mybir, bacc
from concourse.bass_utils import run_bass_kernel_spmd
from concourse.tile import TileContext

# problem constants
N, E, IN, H, H2, OUT = 50000, 600000, 256, 128, 64, 40
EPS, GAMMA = 0.1, 0.1
NCORES = 8
NSH = N // NCORES          # 6250 nodes per core
WCOL = 128                 # dst window width (S chunk width)
NW = (NSH + WCOL - 1) // WCOL      # 49 windows per core (last = 106 cols)
SBW = 4                    # windows per superblock (512 dst cols)
NSB = (NW + SBW - 1) // SBW        # 13 superblocks
HALF = N // 2              # table half split for int16 gather indices
CHUNK = 128
NTILE = (NSH + 127) // 128         # 49 node tiles per core

F32 = mybir.dt.float32
F8 = mybir.dt.float8e4
BF16 = mybir.dt.bfloat16
I16 = mybir.dt.int16
AF = mybir.ActivationFunctionType
ALU = mybir.AluOpType


def _wslice(s):
    lo = s * SBW * WCOL
    hi = min(NSH, (s + 1) * SBW * WCOL)
    return lo, hi - lo


# ----------------------------------------------------------------- prep
def _prep_graph(edge_index):
    src = np.asarray(edge_index[0], dtype=np.int64)
    dst = np.asarray(edge_index[1], dtype=np.int64)
    # GCN norm with self loops included in degrees
    deg = np.bincount(dst, minlength=N).astype(np.float32) + 1.0
    dinv = (1.0 / np.sqrt(deg)).astype(np.float32)
    norm = (dinv[src] * dinv[dst]).astype(np.float32)
    dinv2 = (dinv * dinv).astype(np.float32)       # self-loop coefficients

    core = dst // NSH
    col = dst % NSH
    win = col // WCOL
    src_core = src // NSH
    src_r = src % NSH
    half = (src_r >= NSH // 2).astype(np.int64)
    # half-tables are per-core concatenations: tableA row = core*3125 + r
    tabrow = src_core * (NSH // 2) + (src_r - half * (NSH // 2))

    key = (core * NW + win) * 2 + half
    order = np.argsort(key, kind="stable")          # natural (random) src order kept
    srcS, colS, normS, keyS = tabrow[order], col[order], norm[order], key[order]

    cnt = np.bincount(key, minlength=NCORES * NW * 2).reshape(NCORES, NW, 2)
    CW = np.ceil(cnt / CHUNK).astype(np.int64).max(axis=0)   # [NW, 2] chunks per group
    assert CW.max() * CHUNK <= 1024, CW.max()

    # chunk index layout: for s in sb: for h in (0,1): for w in windows(s): CW[w,h] chunks
    sb_windows = [list(range(s * SBW, min((s + 1) * SBW, NW))) for s in range(NSB)]
    choff_sb = []              # first chunk id of each sb
    blkoff = {}                # (s,h,w) -> block offset within sb (S tile index)
    hblkoff = {}               # (s,h,w) -> block offset within the half's g tile
    ch = 0
    for s, ws in enumerate(sb_windows):
        choff_sb.append(ch)
        boff = 0
        for h in (0, 1):
            hoff = 0
            for w in ws:
                blkoff[(s, h, w)] = boff
                hblkoff[(s, h, w)] = hoff
                nch = int(CW[w, h])
                ch += nch
                boff += nch
                hoff += nch
    TOTCH = ch
    C_sb = [int(sum(CW[w, 0] + CW[w, 1] for w in ws)) for ws in sb_windows]
    CH_sb = [[int(sum(CW[w, h] for w in ws)) for ws in sb_windows] for h in (0, 1)]
    CSB_MAX = max(C_sb)
    CHA_MAX = max(CH_sb[0])
    CHB_MAX = max(CH_sb[1])
    TOT = TOTCH * CHUNK        # gather slots per core per iteration

    grp_start = np.concatenate([[0], np.cumsum(cnt.reshape(-1))]).astype(np.int64)

    idx = np.zeros((NCORES, TOT), dtype=np.int16)
    scoef = np.zeros((NCORES, CHUNK, TOTCH, WCOL), dtype=mybir.dt.np(F8))

    for c in range(NCORES):
        off = 0
        for s, ws in enumerate(sb_windows):
            for h in (0, 1):
                for w in ws:
                    g = (c * NW + w) * 2 + h
                    st, n = grp_start[g], int(cnt[c, w, h])
                    e_src = srcS[st:st + n]
                    e_col = colS[st:st + n]
                    e_nrm = normS[st:st + n]
                    cap = int(CW[w, h])
                    for k in range(cap):
                        lo = k * CHUNK
                        m = max(0, min(CHUNK, n - lo))
                        gch = None
                        # global chunk id for (s,h,w,k)
                        gch = choff_sb[s] + blkoff[(s, h, w)] + k
                        if m > 0:
                            idx[c, off:off + m] = e_src[lo:lo + m]
                            rel = (e_col[lo:lo + m] - w * WCOL).astype(np.int64)
                            scoef[c, np.arange(m), gch, rel] = e_nrm[lo:lo + m]
                        off += CHUNK
        assert off == TOT

    def wrap_idx(flat):
        L = len(flat)
        assert L % 16 == 0
        w16 = flat.reshape(L // 16, 16).T.copy()     # [16, L/16]
        return np.tile(w16, (8, 1))                   # [128, L/16]

    # per-core diagonal chunks: D[n, c] = dinv2[core*NSH + t*128 + n] if n==c
    diagc = np.zeros((NCORES, CHUNK, NTILE, CHUNK), dtype=np.float32)
    for c in range(NCORES):
        for t in range(NTILE):
            tw = min(128, NSH - t * 128)
            v = dinv2[c * NSH + t * 128: c * NSH + t * 128 + tw]
            diagc[c, np.arange(tw), t, np.arange(tw)] = v

    meta = dict(CW=CW, sb_windows=sb_windows, choff_sb=choff_sb, blkoff=blkoff,
                hblkoff=hblkoff, C_sb=C_sb, CSB_MAX=CSB_MAX,
                CHA_MAX=CHA_MAX, CHB_MAX=CHB_MAX, TOTCH=TOTCH, TOT=TOT)
    bf = mybir.dt.np(BF16)
    percore = []
    for c in range(NCORES):
        percore.append(dict(
            idx=wrap_idx(idx[c]).astype(np.int16),
            scoef=scoef[c],
            diagc=diagc[c].astype(bf),
        ))
    return meta, percore


# ---------------------------------------------------------------- build
def _build(meta):
    CW = meta["CW"]
    sb_windows = meta["sb_windows"]
    choff_sb, blkoff = meta["choff_sb"], meta["blkoff"]
    hblkoff = meta["hblkoff"]
    C_sb, CSB_MAX, TOTCH, TOT = meta["C_sb"], meta["CSB_MAX"], meta["TOTCH"], meta["TOT"]
    CHA_MAX, CHB_MAX = meta["CHA_MAX"], meta["CHB_MAX"]

    nc = bacc.Bacc(num_devices=NCORES, num_swdge_queues=4)
    p_xT = nc.declare_dram_parameter("xT", [IN, NSH], F32, isOutput=False)
    p_idx = nc.declare_dram_parameter("idx", [128, TOT // 16], I16, isOutput=False)
    p_sc = nc.declare_dram_parameter("scoef", [128, TOTCH, WCOL], F8, isOutput=False)
    p_diag = nc.declare_dram_parameter("diagc", [128, NTILE, 128], BF16, isOutput=False)
    p_w0T = nc.declare_dram_parameter("w0T", [IN, H], F32, isOutput=False)
    p_b0 = nc.declare_dram_parameter("b0", [H, 1], F32, isOutput=False)
    p_aW1T = nc.declare_dram_parameter("aW1T", [H, H], F32, isOutput=False)
    p_gw1 = nc.declare_dram_parameter("gw1", [H, H], F32, isOutput=False)
    p_ba1 = nc.declare_dram_parameter("ba1", [H, 1], F32, isOutput=False)
    p_w2T = nc.declare_dram_parameter("w2T", [H, H2], F32, isOutput=False)
    p_b2 = nc.declare_dram_parameter("b2", [H2, 1], F32, isOutput=False)
    p_aW2T = nc.declare_dram_parameter("aW2T", [H2, H2], F32, isOutput=False)
    p_gw2 = nc.declare_dram_parameter("gw2", [H2, H2], F32, isOutput=False)
    p_ba2 = nc.declare_dram_parameter("ba2", [H2, 1], F32, isOutput=False)
    p_wfT = nc.declare_dram_parameter("wfT", [H2, OUT], F32, isOutput=False)
    p_bfc = nc.declare_dram_parameter("bfc", [128, OUT], F32, isOutput=False)
    p_ident = nc.declare_dram_parameter("ident", [128, 128], BF16, isOutput=False)
    p_out = nc.declare_dram_parameter("out", [NSH, OUT], F32, isOutput=True)
    p_hd = nc.declare_dram_parameter("hdump", [H, NSH], F32, isOutput=True)
    K_DUMP = os.environ.get("K_DUMP", "")

    ag_ins = [nc.dram_tensor(f"ag_in{i}", [NSH, H], BF16) for i in range(4)]
    tablesA = [nc.dram_tensor(f"tableA{i}", [HALF, H], BF16, addr_space="Shared")
               for i in range(4)]
    tablesB = [nc.dram_tensor(f"tableB{i}", [HALF, H], BF16, addr_space="Shared")
               for i in range(4)]

    with TileContext(nc) as tc:
        with (
            tc.tile_pool(name="const", bufs=1) as cp,
            tc.tile_pool(name="xin", bufs=2) as xp,
            tc.tile_pool(name="gatA", bufs=6) as gpa,
            tc.tile_pool(name="gatB", bufs=3) as gpb,
            tc.tile_pool(name="sstr", bufs=2) as sp,
            tc.tile_pool(name="wrk", bufs=2) as wp,
            tc.tile_pool(name="pa", bufs=3, space="PSUM") as pa,
            tc.tile_pool(name="pt", bufs=2, space="PSUM") as pt,
            tc.tile_pool(name="py", bufs=3, space="PSUM") as py,
        ):
            # ---- persistent state + constants
            hT = cp.tile([H, NSH], F32, tag="hT")
            h2T = cp.tile([H2, NSH], F32, tag="h2T")
            hwT = cp.tile([H, NSH], BF16, tag="hwT")
            rows_all = cp.tile([128, NTILE, 128], BF16, tag="rows_all")
            t_idx = cp.tile([128, TOT // 16], I16, tag="idx")
            t_diag = cp.tile([128, NTILE, 128], BF16, tag="diagc")
            w0a = cp.tile([128, H], F32, tag="w0a")
            w0b = cp.tile([128, H], F32, tag="w0b")
            b0 = cp.tile([H, 1], F32, tag="b0")
            aW1T = cp.tile([H, H], F32, tag="aW1T")
            gw1 = cp.tile([H, H], F32, tag="gw1")
            ba1 = cp.tile([H, 1], F32, tag="ba1")
            w2T = cp.tile([H, H2], F32, tag="w2T")
            b2 = cp.tile([H2, 1], F32, tag="b2")
            aW2T = cp.tile([H2, H2], F32, tag="aW2T")
            gw2 = cp.tile([H2, H2], F32, tag="gw2")
            ba2 = cp.tile([H2, 1], F32, tag="ba2")
            wfT = cp.tile([H2, OUT], F32, tag="wfT")
            bfc = cp.tile([128, OUT], F32, tag="bfc")
            ident = cp.tile([128, 128], BF16, tag="ident")

            nc.sync.dma_start(out=t_idx[:], in_=p_idx[:, :])
            nc.sync.dma_start(out=t_diag[:, :, :], in_=p_diag[:, :, :])
            nc.sync.dma_start(out=w0a[:], in_=p_w0T[0:128, :])
            nc.sync.dma_start(out=w0b[:], in_=p_w0T[128:256, :])
            nc.sync.dma_start(out=b0[:], in_=p_b0[:, :])
            nc.sync.dma_start(out=aW1T[:], in_=p_aW1T[:, :])
            nc.sync.dma_start(out=gw1[:], in_=p_gw1[:, :])
            nc.sync.dma_start(out=ba1[:], in_=p_ba1[:, :])
            nc.sync.dma_start(out=w2T[:], in_=p_w2T[:, :])
            nc.sync.dma_start(out=b2[:], in_=p_b2[:, :])
            nc.sync.dma_start(out=aW2T[:], in_=p_aW2T[:, :])
            nc.sync.dma_start(out=gw2[:], in_=p_gw2[:, :])
            nc.sync.dma_start(out=ba2[:], in_=p_ba2[:, :])
            nc.sync.dma_start(out=wfT[:], in_=p_wfT[:, :])
            nc.sync.dma_start(out=bfc[:], in_=p_bfc[:, :])
            nc.sync.dma_start(out=ident[:], in_=p_ident[:, :])

            # ------------------------------------------------ staging helpers
            def stage_sb(s, src_t, srcdim, gwt, ag_in):
                """hw^T = gwt.T @ src_t for superblock s; write hwT (bf16),
                transpose to rows_all tiles, DMA rows to ag_in."""
                lo, n = _wslice(s)
                ps = pa.tile([srcdim, 512], F32, tag="pa")
                nc.tensor.matmul(ps[:, :n], gwt[:], src_t[:, lo:lo + n],
                                 start=True, stop=True)
                nc.scalar.activation(hwT[0:srcdim, lo:lo + n], ps[:, :n], AF.Copy)
                nt = (n + 127) // 128
                for t in range(nt):
                    tile = s * SBW + t
                    tw = min(128, n - t * 128)
                    ptt = pt.tile([128, 128], BF16, tag="pt")
                    nc.tensor.transpose(ptt[:tw, :srcdim],
                                        hwT[0:srcdim, lo + t * 128: lo + t * 128 + tw],
                                        ident[:srcdim, :srcdim])
                    nc.scalar.activation(rows_all[:tw, tile, 0:srcdim],
                                         ptt[:tw, :srcdim], AF.Copy)
                    nc.sync.dma_start(
                        out=ag_in[lo + t * 128: lo + t * 128 + tw, 0:srcdim],
                        in_=rows_all[:tw, tile, 0:srcdim])

            def fire_ag(it, part):
                if part == 0:
                    nc.gpsimd.collective_compute(
                        "AllGather", ALU.bypass,
                        replica_groups=[list(range(NCORES))],
                        ins=[ag_ins[it][0:NSH // 2, :]], outs=[tablesA[it][:, :]])
                else:
                    nc.gpsimd.collective_compute(
                        "AllGather", ALU.bypass,
                        replica_groups=[list(range(NCORES))],
                        ins=[ag_ins[it][NSH // 2:NSH, :]], outs=[tablesB[it][:, :]])

            # ------------------------------------------------ gather issue
            qn_state = [0]

            def gather_group(g_tiles, it, s, h):
                """issue gathers for (superblock s, half h)."""
                tab = (tablesA if h == 0 else tablesB)[it]
                g = g_tiles[(s, h)]
                for w in sb_windows[s]:
                    cap = int(CW[w, h])
                    if cap == 0:
                        continue
                    b0_ = hblkoff[(s, h, w)]
                    gch = choff_sb[s] + blkoff[(s, h, w)]
                    o = gch * CHUNK          # global slot offset
                    nidx = cap * CHUNK
                    nc.gpsimd.dma_gather(
                        out_ap=g[:, b0_:b0_ + cap, :], in_ap=tab[:, :],
                        idxs_ap=t_idx[:, o // 16:(o + nidx) // 16],
                        num_idxs=nidx, num_idxs_reg=nidx,
                        elem_size=H, queue_num=qn_state[0] % 4)
                    qn_state[0] += 1

            # ------------------------------------------------ psy compute
            def psy_sb(s, state_t, dim, aWt, bias_t, gA, gB):
                """accumulate psy for superblock s, then tanh+update state."""
                lo, n = _wslice(s)
                st_ = sp.tile([128, CSB_MAX, WCOL], F8, tag="sg")
                nc.scalar.dma_start(
                    out=st_[:, 0:C_sb[s], :],
                    in_=p_sc[:, choff_sb[s]:choff_sb[s] + C_sb[s], :])
                psy = py.tile([dim, 512], F32, tag="py")
                first = True
                for h, gh in ((0, gA), (1, gB)):
                    for w in sb_windows[s]:
                        wl = w - s * SBW
                        wn = min(WCOL, NSH - w * WCOL)
                        for k in range(int(CW[w, h])):
                            blk = blkoff[(s, h, w)] + k
                            hblk = hblkoff[(s, h, w)] + k
                            nc.tensor.matmul(
                                psy[:, wl * WCOL: wl * WCOL + wn],
                                gh[:, hblk, 0:dim], st_[:, blk, 0:wn],
                                start=first, stop=False, skip_group_check=True)
                            first = False
                # self-loop diagonal terms
                nt = (n + 127) // 128
                for t in range(nt):
                    tile = s * SBW + t
                    tw = min(128, n - t * 128)
                    nc.tensor.matmul(
                        psy[:, t * 128: t * 128 + tw],
                        rows_all[0:tw, tile, 0:dim],
                        t_diag[0:tw, tile, 0:tw],
                        start=False, stop=False, skip_group_check=True)
                # aW term
                nc.tensor.matmul(psy[:, :n], aWt[:], state_t[:, lo:lo + n],
                                 start=False, stop=True, skip_group_check=True)
                upd = wp.tile([dim, 512], F32, tag="upd")
                nc.scalar.activation(upd[:, :n], psy[:, :n], AF.Tanh, bias=bias_t[:, :])
                nc.vector.scalar_tensor_tensor(
                    state_t[:, lo:lo + n], upd[:, :n], EPS,
                    state_t[:, lo:lo + n], ALU.mult, ALU.add)

            # ------------------------------------------------ final per sb
            def final_sb(s):
                lo, n = _wslice(s)
                nt = (n + 127) // 128
                for t in range(nt):
                    t0_ = lo + t * 128
                    tw = min(128, n - t * 128)
                    pf = pa.tile([128, 512], F32, tag="pa")
                    nc.tensor.matmul(pf[:tw, :OUT], h2T[:, t0_:t0_ + tw],
                                     wfT[:], start=True, stop=True)
                    lg = wp.tile([128, OUT], F32, tag="lg")
                    nc.vector.tensor_tensor(lg[:tw, :], pf[:tw, :OUT], bfc[:tw, :], ALU.add)
                    nmx = wp.tile([128, 1], F32, tag="nmx")
                    nc.vector.tensor_reduce(nmx[:tw, :], lg[:tw, :],
                                            mybir.AxisListType.X, ALU.max, negate=True)
                    ex = wp.tile([128, OUT], F32, tag="ex")
                    se = wp.tile([128, 1], F32, tag="se")
                    nc.scalar.activation(ex[:tw, :], lg[:tw, :], AF.Exp,
                                         bias=nmx[:tw, :], accum_out=se[:tw, :])
                    lse = wp.tile([128, 1], F32, tag="lse")
                    nc.scalar.activation(lse[:tw, :], se[:tw, :], AF.Ln)
                    shift = wp.tile([128, 1], F32, tag="shift")
                    nc.vector.tensor_tensor(shift[:tw, :], nmx[:tw, :], lse[:tw, :],
                                            ALU.subtract)
                    ot = wp.tile([128, OUT], F32, tag="ot")
                    nc.vector.tensor_scalar(ot[:tw, :], lg[:tw, :], shift[:tw, :],
                                            None, ALU.add)
                    nc.sync.dma_start(out=p_out[t0_:t0_ + tw, :], in_=ot[:tw, :])

            # ================================================ layer 0 (+T0 staging)
            for s in range(NSB):
                lo, n = _wslice(s)
                ps = pa.tile([H, 512], F32, tag="pa")
                for kc, w0t in enumerate((w0a, w0b)):
                    xt = xp.tile([128, 512], F32, tag="xt")
                    nc.sync.dma_start(out=xt[:, :n],
                                      in_=p_xT[kc * 128:(kc + 1) * 128, lo:lo + n])
                    nc.tensor.matmul(ps[:, :n], w0t[:], xt[:, :n],
                                     start=(kc == 0), stop=(kc == 1))
                t0_ = wp.tile([H, 512], F32, tag="upd")
                nc.scalar.activation(t0_[:, :n], ps[:, :n], AF.Identity, bias=b0[:, :])
                nc.vector.scalar_tensor_tensor(hT[:, lo:lo + n], t0_[:, :n], 0.01,
                                               t0_[:, :n], ALU.mult, ALU.max)
                stage_sb(s, hT, H, gw1, ag_ins[0])
                if s == 6:
                    fire_ag(0, 0)
            fire_ag(0, 1)
            if K_DUMP == "h0":
                nc.sync.dma_start(out=p_hd[:, :], in_=hT[:, :])

            # ================================================ conv iterations
            def conv_iteration(it, state_t, dim, aWt, bias_t, gwt_next, post):
                """post(s) runs after update of superblock s (staging for the
                next phase); gathers are pipelined A-ahead."""
                g_tiles = {}
                for s in range(NSB):
                    g_tiles[(s, 0)] = gpa.tile([128, CHA_MAX, 128], BF16, tag="ga",
                                               name=f"ga_{it}_{s}")
                    g_tiles[(s, 1)] = gpb.tile([128, CHB_MAX, 128], BF16, tag="gb",
                                               name=f"gb_{it}_{s}")
                # prologue: A gathers several superblocks ahead
                for s0 in range(min(6, NSB)):
                    gather_group(g_tiles, it, s0, 0)
                gather_group(g_tiles, it, 0, 1)
                for s in range(NSB):
                    if s + 1 < NSB:
                        gather_group(g_tiles, it, s + 1, 1)
                    if s + 6 < NSB:
                        gather_group(g_tiles, it, s + 6, 0)
                    psy_sb(s, state_t, dim, aWt, bias_t, g_tiles[(s, 0)], g_tiles[(s, 1)])
                    post(s)

            # ---- conv1 iter 0 (stage T1)
            def post0(s):
                stage_sb(s, hT, H, gw1, ag_ins[1])
                if s == 6:
                    fire_ag(1, 0)
                if s == NSB - 1:
                    fire_ag(1, 1)
            conv_iteration(0, hT, H, aW1T, ba1, gw1, post0)
            if K_DUMP == "it1":
                nc.sync.dma_start(out=p_hd[:, :], in_=hT[:, :])

            # ---- conv1 iter 1 (stage T2)
            def post1(s):
                stage_sb(s, hT, H, gw1, ag_ins[2])
                if s == 6:
                    fire_ag(2, 0)
                if s == NSB - 1:
                    fire_ag(2, 1)
            conv_iteration(1, hT, H, aW1T, ba1, gw1, post1)
            if K_DUMP == "it2":
                nc.sync.dma_start(out=p_hd[:, :], in_=hT[:, :])

            # ---- conv1 iter 2 (transition + stage T3 from h2T)
            def post2(s):
                lo, n = _wslice(s)
                gk = wp.tile([H, 512], F32, tag="gk")
                nc.vector.scalar_tensor_tensor(gk[:, :n], hT[:, lo:lo + n], 0.01,
                                               hT[:, lo:lo + n], ALU.mult, ALU.max)
                ps = pa.tile([H2, 512], F32, tag="pa")
                nc.tensor.matmul(ps[:, :n], w2T[:], gk[:, :n], start=True, stop=True)
                t2 = wp.tile([H2, 512], F32, tag="upd")
                nc.scalar.activation(t2[:, :n], ps[:, :n], AF.Identity, bias=b2[:, :])
                nc.vector.scalar_tensor_tensor(h2T[:, lo:lo + n], t2[:, :n], 0.01,
                                               t2[:, :n], ALU.mult, ALU.max)
                stage_sb(s, h2T, H2, gw2, ag_ins[3])
                if s == 6:
                    fire_ag(3, 0)
                if s == NSB - 1:
                    fire_ag(3, 1)
            conv_iteration(2, hT, H, aW1T, ba1, gw1, post2)
            if K_DUMP == "it3":
                nc.sync.dma_start(out=p_hd[:, :], in_=hT[:, :])

            # ---- conv2 (final per sb)
            conv_iteration(3, h2T, H2, aW2T, ba2, None, final_sb)
            if K_DUMP == "h2":
                nc.sync.dma_start(out=p_hd[:64, :], in_=h2T[:, :])

    nc.finalize()
    return nc


# ----------------------------------------------------------------- run
def kernel(x, edge_index, w_hid, b_hid, W_a1, gcn_w1, b_a1,
           w_hid2, b_hid2, W_a2, gcn_w2, b_a2, w_fc, b_fc, _trace=False):
    x = np.asarray(x, np.float32)
    meta, percore = _prep_graph(edge_index)
    nc = _build(meta)

    f32 = np.float32
    bf = mybir.dt.np(BF16)
    w0T = np.ascontiguousarray(np.asarray(w_hid, f32).T)            # [256,128]
    aW1 = np.asarray(W_a1, f32)
    aW1T = np.ascontiguousarray(aW1.T - aW1 - GAMMA * np.eye(H, dtype=f32))
    aW2 = np.asarray(W_a2, f32)
    aW2T = np.ascontiguousarray(aW2.T - aW2 - GAMMA * np.eye(H2, dtype=f32))
    common = dict(
        w0T=w0T,
        b0=np.asarray(b_hid, f32).reshape(H, 1),
        aW1T=aW1T,
        gw1=np.ascontiguousarray(np.asarray(gcn_w1, f32)),
        ba1=np.asarray(b_a1, f32).reshape(H, 1),
        w2T=np.ascontiguousarray(np.asarray(w_hid2, f32).T),
        b2=np.asarray(b_hid2, f32).reshape(H2, 1),
        aW2T=aW2T,
        gw2=np.ascontiguousarray(np.asarray(gcn_w2, f32)),
        ba2=np.asarray(b_a2, f32).reshape(H2, 1),
        wfT=np.ascontiguousarray(np.asarray(w_fc, f32).T),
        bfc=np.tile(np.asarray(b_fc, f32).reshape(1, OUT), (128, 1)),
        ident=np.eye(128, dtype=bf),
    )
    in_maps = []
    for c in range(NCORES):
        xT = np.ascontiguousarray(x[c * NSH:(c + 1) * NSH].T)
        in_maps.append({"xT": xT, **percore[c], **common})

    res = run_bass_kernel_spmd(nc, in_maps, list(range(NCORES)), trace=_trace)
    out = np.concatenate([res.results[c]["out"] for c in range(NCORES)], axis=0)
    kernel.last_hdump = np.stack([res.results[c]["hdump"] for c in range(NCORES)])
    kernel.last_exec_time_ns = res.exec_time_ns
    kernel.last_results = res
    return out
